# revision 26
# baseline (speedup 1.0000x reference)
"""Trainium2 Bass kernel for nn_CudaMixedBitLinear (GPTQ-style 4-bit linear).

out[b,s,o] = sum_k x[b,s,k] * W[o,k],  W[o,k] = (q[o,k] - z[o,g]) * s[o,g],
g = k // 128, q/z packed as nibbles (low nibble first) in int32 bytes.

Sharding: column-parallel over out_features across 8 cores (11008 -> 1376
per core), x replicated, outputs concatenated on host. No collectives.

v2 design (no PE transposes; CoreSim 593.8us vs 643.0us for the
transpose-based v1): the packed weights are shipped TRANSPOSED
([K/2, OC] uint16 holding the 0..255 byte values - a host layout/widening
cast), so dequant happens directly in the W^T orientation the matmuls
need. Within each 256-wide k block the k order is permuted to (even k's |
odd k's) so the low/high nibble planes of one byte-row become two whole
128-partition k-tiles; x^T gets the same permutation on the host
(layout-only), which leaves the GEMM result unchanged. With that
permutation each k-tile pair spans exactly groups (2t, 2t+1) split at
partition 64, so the per-(o,group) scales and zero points enter as
[128, OC] tiles replicated on the host (scales: layout-only; qzeros: the
16x11008 packed zero bytes are unpacked to fp16(1024+z) host-side - tiny
metadata prep, <1% of input bytes - the 45M-element weight dequant and
all GEMM arithmetic stay on device).

Per-core device schedule:
  - PE warmup: ~3us of dep-free dummy matmuls start the P-state/HAM ramp.
  - dequant, two slice-major passes (wt cols 0:512 for all 16 pairs, then
    512:1376): au = (q16 & 15) | 0x6400 (= fp16 bits of 1024+q_even),
    bu = (q16 >> 4) | 0x6400, then wt = (au_fp16 - zb) * sb - exact until
    the single fp16 round of (q-z)*s. All on DVE except the B-subtract
    (GPSIMD) so the DVE feed rate stays under the PE JIT consumption rate.
  - PE runs matmuls just-in-time, chunk-major (o-chunks 512/512/352): for
    each chunk, m-subtiles 0-7 accumulate pair (2t, 2t+1) right after pair
    t's slice dequants - 8 PSUM banks hold 8 subtile accumulators, retiring
    per chunk. Chunk 2 runs subtile-major so early x buffers free up for
    the steady-state block prefetch.
  - steady state: m-blocks 4-15 (XB=256, ring of 4 x buffers), 3-chunk
    PSUM accumulation per subtile, ACT drains, per-chunk DMA out; the last
    subtile runs chunk-outer with a narrow (128-col) final group so the
    end-of-kernel tail is one small DVE copy + SP DMA.
DMA placement: x loads + first qweight tile + final drain on SP HWDGE;
qweight/scale tiles on ACT HWDGE (their readers are DVE-only, so the
single-sync-wait HWDGE limit holds); zero tiles (read by DVE and GPSIMD)
and output stores on GPSIMD SWDGE, which allows multiple waits.
"""

import numpy as np

B, S, K = 2, 2048, 4096
OUT_F = 11008
N_CORES = 8
OC = OUT_F // N_CORES       # 1376 out features per core
M = B * S                   # 4096 rows
KT = K // 128               # 32 k-tiles (permuted order)
PAIRS = KT // 2             # 16 nibble pairs
CH = [(0, 512), (512, 1024), (1024, OC)]
XB = 256                    # m columns per x block
NBLK = M // XB              # 16 m-blocks
XPARTS = 4                  # per-block x load split (8 k-tiles each)
AJ_SUBS = 8                 # phase-A JIT m-subtiles (blocks 0-3)

_CACHE = {}
RUN_KWARGS = {}   # test harness can inject e.g. dict(trace=True)
LAST_RESULT = None


def _build_bass(loop_r=None):
    # loop_r: wrap the whole per-core program in an in-NEFF hardware loop.
    # WARNING: the looped variant compiles but wedges the device at runtime
    # (NRT_EXEC_UNIT_UNRECOVERABLE); kept only as a record of the attempt.
    # Normal operation (loop_r=None) is unaffected.
    import concourse.bass as bass
    import concourse.bacc as bacc
    import concourse.mybir as mybir
    from concourse.tile import TileContext

    A = mybir.AluOpType
    fp16 = mybir.dt.float16
    f32 = mybir.dt.float32
    u16 = mybir.dt.uint16

    nc = bacc.Bacc("TRN2", target_bir_lowering=False)
    xt = nc.dram_tensor("xt", [K, M], fp16, kind="ExternalInput")
    qwt = nc.dram_tensor("qwt", [K // 2, OC], u16, kind="ExternalInput")
    sbb = nc.dram_tensor("sbb", [PAIRS, 128, OC], fp16, kind="ExternalInput")
    zbb = nc.dram_tensor("zbb", [PAIRS, 128, OC], fp16, kind="ExternalInput")
    out = nc.dram_tensor("out", [M, OC], f32, kind="ExternalOutput")

    wt = nc.alloc_sbuf_tensor("wt", [128, KT * OC], fp16).ap()
    xts = [nc.alloc_sbuf_tensor(f"xtbuf{i}", [128, KT, XB], fp16).ap()
           for i in range(4)]

    xt_view = xt[:, :].rearrange("(kt p) m -> p kt m", p=128)  # [128, KT, M]
    wt3 = wt.rearrange("p (kt oc) -> p kt oc", kt=KT)

    def xpart_load(b, slot):
        for P in range(XPARTS):
            kp = KT // XPARTS
            nc.sync.dma_start(
                out=xts[slot][:, P * kp:(P + 1) * kp, :],
                in_=xt_view[:, P * kp:(P + 1) * kp, XB * b:XB * (b + 1)])

    from contextlib import ExitStack
    with TileContext(nc) as tc:
        with (
            tc.tile_pool(name="deq", bufs=2) as deq,
            tc.tile_pool(name="obp", bufs=4) as obp,
            tc.tile_pool(name="acc", bufs=1, space="PSUM") as accp,
            ExitStack() as _es,
        ):
            if loop_r is not None:
                _es.enter_context(tc.For_i(0, loop_r))
            # PE warmup: dep-free dummy matmuls start the P-state ramp
            # (and the HAM activity window on hardware) ~3us before the
            # first real matmul; results are never read and the first real
            # accumulation into this bank resets it via start=True
            dmy = deq.tile([128, 64], fp16, tag="dmy", name="dmy")
            nc.vector.memset(dmy, 0.0)
            wrm = accp.tile([128, 512], f32, tag="s7", name="wrm")
            for _ in range(45):
                nc.tensor.matmul(wrm[0:64, 0:64], lhsT=dmy, rhs=dmy,
                                 start=True, stop=True)

            # pair-0 chunk-0 packed weights ride SP first: shortest DGE
            # chain, so the very first dequant starts as early as possible
            qt00 = deq.tile([128, 512], u16, tag="qt0", name="qt0")
            nc.sync.dma_start(out=qt00, in_=qwt[0:128, 0:512])

            # x blocks 0-3 for the phase-A JIT window: a small 2-k-tile
            # "head" per block first (the DMA pipe serializes transfers, so
            # full first parts would make blocks 1-3 miss the first JIT
            # matmuls), then the bulk in part-major order
            for b in range(4):
                nc.sync.dma_start(out=xts[b][:, 0:2, :],
                                  in_=xt_view[:, 0:2, XB * b:XB * (b + 1)])
            for P0, P1 in ((2, 12), (12, 22), (22, 32)):
                for b in range(4):
                    nc.sync.dma_start(
                        out=xts[b][:, P0:P1, :],
                        in_=xt_view[:, P0:P1, XB * b:XB * (b + 1)])

            # ------------- dequant (two passes, slice-major) -------------
            # pass 0 covers wt columns 0:512 (all the c0-phase JIT needs)
            # for every pair first, so the PE feed is never o-column-starved;
            # pass 1 fills columns 512:OC in time for the c1/c2 phases.
            def deq_pass(t, d0, d1, sfx):
                w = d1 - d0
                if t == 0 and sfx == "0":
                    qt = qt00
                else:
                    qt = deq.tile([128, w], u16, tag="qt" + sfx,
                                  name="qt" + sfx)
                    nc.scalar.dma_start(
                        out=qt, in_=qwt[t * 128:(t + 1) * 128, d0:d1])
                sb = deq.tile([128, w], fp16, tag="sb" + sfx, name="sb" + sfx)
                nc.scalar.dma_start(out=sb, in_=sbb[t, :, d0:d1])
                zb = deq.tile([128, w], fp16, tag="zb" + sfx, name="zb" + sfx)
                nc.gpsimd.dma_start(out=zb, in_=zbb[t, :, d0:d1])

                au = deq.tile([128, w], u16, tag="au" + sfx, name="au" + sfx)
                bu = deq.tile([128, w], u16, tag="bu" + sfx, name="bu" + sfx)
                amz = deq.tile([128, w], fp16, tag="am" + sfx, name="am" + sfx)
                bmz = deq.tile([128, w], fp16, tag="bm" + sfx, name="bm" + sfx)
                if t == 0 and sfx == "0":
                    # startup ordering: full A chain first - the very first
                    # matmuls need only wt[kt=0]; B follows immediately
                    nc.vector.tensor_scalar(out=au, in0=qt, scalar1=15,
                                            scalar2=0x6400,
                                            op0=A.bitwise_and,
                                            op1=A.bitwise_or)
                    nc.vector.tensor_tensor(out=amz, in0=au.bitcast(fp16),
                                            in1=zb, op=A.subtract)
                    nc.vector.tensor_tensor(out=wt3[:, 2 * t, d0:d1],
                                            in0=amz, in1=sb, op=A.mult)
                    nc.vector.tensor_scalar(out=bu, in0=qt, scalar1=4,
                                            scalar2=0x6400,
                                            op0=A.logical_shift_right,
                                            op1=A.bitwise_or)
                    nc.vector.tensor_tensor(out=bmz, in0=bu.bitcast(fp16),
                                            in1=zb, op=A.subtract)
                    nc.vector.tensor_tensor(out=wt3[:, 2 * t + 1, d0:d1],
                                            in0=bmz, in1=sb, op=A.mult)
                    return
                nc.vector.tensor_scalar(out=au, in0=qt, scalar1=15,
                                        scalar2=0x6400, op0=A.bitwise_and,
                                        op1=A.bitwise_or)
                nc.vector.tensor_tensor(out=amz, in0=au.bitcast(fp16),
                                        in1=zb, op=A.subtract)
                nc.vector.tensor_tensor(out=wt3[:, 2 * t, d0:d1], in0=amz,
                                        in1=sb, op=A.mult)
                nc.vector.tensor_scalar(out=bu, in0=qt, scalar1=4,
                                        scalar2=0x6400,
                                        op0=A.logical_shift_right,
                                        op1=A.bitwise_or)
                seng = nc.vector if t == 0 else nc.gpsimd
                seng.tensor_tensor(out=bmz, in0=bu.bitcast(fp16),
                                   in1=zb, op=A.subtract)
                nc.vector.tensor_tensor(out=wt3[:, 2 * t + 1, d0:d1],
                                        in0=bmz, in1=sb, op=A.mult)

            for t in range(PAIRS):
                deq_pass(t, 0, 512, "0")
            for t in range(PAIRS):
                deq_pass(t, 512, OC, "1")

            # ------------- phase-A JIT GEMM (chunk-major) -------------
            def jit_mm(accs, s, kt_, c0, c1, start, stop):
                nc.tensor.matmul(
                    accs[s][:, :c1 - c0],
                    lhsT=xts[s // 2][:, kt_, (s % 2) * 128:(s % 2) * 128 + 128],
                    rhs=wt[:, kt_ * OC + c0: kt_ * OC + c1],
                    start=start, stop=stop)

            def drain(accs, s, c0, c1):
                ob = obp.tile([128, 512], f32, tag="ob", name="ob")
                nc.scalar.copy(out=ob[:, :c1 - c0], in_=accs[s][:, :c1 - c0])
                nc.gpsimd.dma_start(out=out[s * 128:(s + 1) * 128, c0:c1],
                                    in_=ob[:, :c1 - c0])

            for ci, (c0, c1) in enumerate(CH):
                accs = {s: accp.tile([128, 512], f32, tag=f"s{s}", name=f"s{s}")
                        for s in range(AJ_SUBS)}
                if ci < 2:
                    # pair-major: consume pair t right after its dequant;
                    # kt-outer so the 8 even-kt matmuls overlap the odd
                    # k-tile's dequant completing
                    for t in range(PAIRS):
                        for s in range(AJ_SUBS):
                            jit_mm(accs, s, 2 * t, c0, c1, t == 0, False)
                        for s in range(AJ_SUBS):
                            jit_mm(accs, s, 2 * t + 1, c0, c1, False,
                                   t == PAIRS - 1)
                    for s in range(AJ_SUBS):
                        drain(accs, s, c0, c1)
                else:
                    # chunk 2: subtile-major so blocks 0/1 retire early and
                    # the steady-state x prefetch overlaps the remaining JIT
                    for s in range(AJ_SUBS):
                        for kt_ in range(KT):
                            jit_mm(accs, s, kt_, c0, c1, kt_ == 0, kt_ == KT - 1)
                        drain(accs, s, c0, c1)
                        if s == 1:
                            xpart_load(4, 0)
                        elif s == 3:
                            xpart_load(5, 1)

            # ---------------- steady state: blocks 4-15 ----------------
            for b in range(4, NBLK):
                if b + 2 < NBLK:
                    xpart_load(b + 2, (b + 2) % 4)
                for sh in range(2):
                    s = 2 * b + sh
                    last = (b == NBLK - 1 and sh == 1)
                    accs = {}
                    if not last:
                        for j, (c0, c1) in enumerate(CH):
                            accs[j] = accp.tile([128, 512], f32,
                                                tag=f"s{2 * j + (s % 2)}",
                                                name=f"st{j}")
                        for kt_ in range(KT):
                            for j, (c0, c1) in enumerate(CH):
                                nc.tensor.matmul(
                                    accs[j][:, :c1 - c0],
                                    lhsT=xts[b % 4][:, kt_,
                                                    sh * 128:sh * 128 + 128],
                                    rhs=wt[:, kt_ * OC + c0: kt_ * OC + c1],
                                    start=kt_ == 0, stop=kt_ == KT - 1)
                        for j, (c0, c1) in enumerate(CH):
                            ob = obp.tile([128, 512], f32, tag="ob", name="ob")
                            nc.scalar.copy(out=ob[:, :c1 - c0],
                                           in_=accs[j][:, :c1 - c0])
                            nc.gpsimd.dma_start(
                                out=out[s * 128:(s + 1) * 128, c0:c1],
                                in_=ob[:, :c1 - c0])
                    else:
                        # last subtile: chunk-outer with a narrow final
                        # accumulation group so the end-of-kernel serial
                        # tail is one small ACT copy + one small SP DMA
                        lch = [(0, 512, 1), (512, 1024, 3),
                               (1024, 1248, 5), (1248, OC, 7)]
                        for c0, c1, tg in lch:
                            acc = accp.tile([128, 512], f32, tag=f"s{tg}",
                                            name=f"lt{tg}")
                            for kt_ in range(KT):
                                nc.tensor.matmul(
                                    acc[:, :c1 - c0],
                                    lhsT=xts[b % 4][:, kt_,
                                                    sh * 128:sh * 128 + 128],
                                    rhs=wt[:, kt_ * OC + c0: kt_ * OC + c1],
                                    start=kt_ == 0, stop=kt_ == KT - 1)
                            ob = obp.tile([128, 512], f32, tag="ob",
                                          name="ob")
                            if c1 == OC:
                                # final piece: copy on the idle DVE (shorter
                                # PSUM-access init than ACT), DMA on idle SP
                                nc.vector.tensor_copy(out=ob[:, :c1 - c0],
                                                      in_=acc[:, :c1 - c0])
                                nc.sync.dma_start(
                                    out=out[s * 128:(s + 1) * 128, c0:c1],
                                    in_=ob[:, :c1 - c0])
                            else:
                                nc.scalar.copy(out=ob[:, :c1 - c0],
                                               in_=acc[:, :c1 - c0])
                                nc.gpsimd.dma_start(
                                    out=out[s * 128:(s + 1) * 128, c0:c1],
                                    in_=ob[:, :c1 - c0])

    if not nc.is_finalized():
        nc.finalize()
    return nc


def kernel(x, qweight, scales, qzeros, group_size=128, **_unused):
    global LAST_RESULT
    from concourse.bass_utils import run_bass_kernel_spmd

    if "nc" not in _CACHE:
        _CACHE["nc"] = _build_bass()
    nc = _CACHE["nc"]

    x2d = np.asarray(x).reshape(M, K)
    # k-permuted x^T: within each 256-block, even k's first then odd k's,
    # matching the nibble planes of the transposed packed weights
    xT = np.ascontiguousarray(x2d.T)                       # [K, M]
    xtp = np.ascontiguousarray(
        xT.reshape(PAIRS, 128, 2, M).transpose(0, 2, 1, 3)).reshape(K, M)

    qweight = np.asarray(qweight)
    scales = np.asarray(scales)
    qzeros = np.asarray(qzeros)

    in_maps = []
    for i in range(N_CORES):
        sl = slice(i * OC, (i + 1) * OC)
        qwc = np.ascontiguousarray(qweight[sl].astype(np.uint16).T)  # [K/2, OC]

        scc = np.asarray(scales[sl], dtype=np.float16)              # [OC, 32]
        sbb = np.empty((PAIRS, 128, OC), np.float16)
        sbb[:, 0:64, :] = scc[:, 0::2].T[:, None, :]
        sbb[:, 64:128, :] = scc[:, 1::2].T[:, None, :]

        qzc = qzeros[sl]                                            # [OC, 16]
        zlo = (1024 + (qzc & 15)).astype(np.float16)                # exact
        zhi = (1024 + ((qzc >> 4) & 15)).astype(np.float16)
        zbb = np.empty((PAIRS, 128, OC), np.float16)
        zbb[:, 0:64, :] = zlo.T[:, None, :]
        zbb[:, 64:128, :] = zhi.T[:, None, :]

        in_maps.append({
            "xt": xtp,
            "qwt": qwc,
            "sbb": np.ascontiguousarray(sbb),
            "zbb": np.ascontiguousarray(zbb),
        })

    res = run_bass_kernel_spmd(nc, in_maps, core_ids=list(range(N_CORES)),
                               **RUN_KWARGS)
    LAST_RESULT = res
    outs = [r["out"] for r in res.results]
    return np.concatenate(outs, axis=1).reshape(B, S, OUT_F).astype(np.float32)
